# revision 1
# baseline (speedup 1.0000x reference)
"""CQAttention (QANet context-query attention) Trainium2 kernel, v5 (bf16).

Full-input contract: kernel(**inputs) takes the unsharded arrays
  C [64, 1024, 256] f32, Q [64, 128, 256] f32,
  cmask [64, 1024] f32 (unused by the reference), qmask [64, 128] f32,
  w [768] f32
and returns out [64, 1024, 512] f32.

Sharding: batch dim across 8 NeuronCores (8 batches per core), no
cross-core communication.

Math notes (vs the reference):
  S[b,i,j] = C@w1 + Q@w2 + (C*w3)@Q^T, masked over j, softmax over j.
  - C@w1 is constant along the softmax axis j -> dropped (w1 unused).
  - q2 = Q@w2 is folded into the exp as a per-partition bias:
    bias = q2 - 1e4*qmask, so masked columns give exp(x-1e4) == 0.0
    exactly (underflow), identical to -1e30 mask + softmax.
  - Softmax denominator s[i] = sum_j E[j,i] via separate N=1 matmuls
    against a ones column, batched 4-per-PSUM-bank so one reciprocal
    op covers a half-batch.

Perf notes:
  - fp32r matmuls execute in fp32 HIGH (4-pump) mode on HW, so the
    whole matmul path is bf16 (1 cycle/row, FWL weight loads).
    rel err ~4e-3 vs the 2e-2 gate.
  - C^T via plain matmuls against a bf16 identity (~107ns spacing,
    counts as PE-busy for the HAM clock gate; transpose-mode does
    not and runs 2.5x slower).
  - Kernel is DMA-floor-bound: ~26MB @ ~360-400GB/s/core => ~70us.
    Everything else (engine schedule below) exists to keep the
    per-batch compute period at or below the store-drain period.
  - DMA: small inputs FIRST on the SP ring (Q batch 0 before all so
    qprep(0) unblocks at ~8us; 1KB-descriptor DMAs starve behind
    queued 8KB C loads in the DMA-engine round-robin). C loads 3
    deep, then pipelined b+3 (issuing all 8 up front exhausts the
    DMA semaphore pool and serializes issue at ~34us). Stores:
    batches 0-4 on the ACT ring, 5-7 on the then-idle SP ring, last
    batch in halves to shorten the drain tail.
  - Engine schedule per batch (measured ns budgets):
    ACT : cast C->bf16 (2x1.15u), exp (2x0.63), A-scale tt0/tt2
          (4x0.56), 1 ct-copy (0.69), store issue
    DVE : 3 ct-copies, recip (2x0.12), A-scale tt1/tt3, fused
          C*A=(U*r)*C from PSUM tt2/tt3, qT-copy, q2, bias
    POOL: C*A=A*C tt0/tt1 (SBUF only -- GPSIMD cannot touch PSUM),
          qw3T scale, q_rnd cast
"""

from contextlib import ExitStack

import numpy as np

import concourse.bacc as bacc
import concourse.bass as bass
import concourse.mybir as mybir
import concourse.tile as tile
from concourse.bass_utils import run_bass_kernel_spmd
from concourse.masks import make_identity

B, LC, LQ, D = 64, 1024, 128, 256
N_CORES = 8
BL = B // N_CORES  # batches per core
NT = LC // 128     # i-chunks per batch
KD = D // 128      # d-chunks (contraction tiles)
F32 = mybir.dt.float32
BF16 = mybir.dt.bfloat16
MULT = mybir.AluOpType.mult

_CACHE: dict = {}


def _build_bass() -> bass.Bass:
    nc = bacc.Bacc("TRN2")
    C_h = nc.dram_tensor("C", [BL, LC, D], F32, kind="ExternalInput")
    Q_h = nc.dram_tensor("Q", [BL, LQ, D], F32, kind="ExternalInput")
    qm_h = nc.dram_tensor("qmask", [BL, LQ], F32, kind="ExternalInput")
    w_h = nc.dram_tensor("w", [3 * D], F32, kind="ExternalInput")
    out_h = nc.dram_tensor("out", [BL, LC, 2 * D], F32, kind="ExternalOutput")

    with tile.TileContext(nc) as tc, ExitStack() as ctx:
        singles = ctx.enter_context(tc.tile_pool(name="singles", bufs=1))
        c_pool = ctx.enter_context(tc.tile_pool(name="c", bufs=BL))
        cb_pool = ctx.enter_context(tc.tile_pool(name="cb", bufs=2))
        ct_pool = ctx.enter_context(tc.tile_pool(name="ct", bufs=2))
        e_pool = ctx.enter_context(tc.tile_pool(name="e", bufs=2))
        # bufs=3: with 2, batch b+2's epilogue stalls on store(b)'s 2MB
        # drain (observed as a ~5us all-engine gap per batch)
        o_pool = ctx.enter_context(tc.tile_pool(name="o", bufs=3))
        small_pool = ctx.enter_context(tc.tile_pool(name="small", bufs=12))
        scratch_pool = ctx.enter_context(tc.tile_pool(name="scr", bufs=2))
        # PSUM budget (8 banks): ctp 2 + s 2 + u 3 + sd 1 = 8
        ctp_pool = ctx.enter_context(tc.tile_pool(name="ctp", bufs=2, space="PSUM"))
        s_pool = ctx.enter_context(tc.tile_pool(name="s", bufs=2, space="PSUM"))
        u_pool = ctx.enter_context(tc.tile_pool(name="u", bufs=3, space="PSUM"))
        sd_pool = ctx.enter_context(tc.tile_pool(name="sd", bufs=1, space="PSUM"))

        # ---------------- one-time setup ----------------
        ident32 = singles.tile([128, 128], F32)
        make_identity(nc, ident32)
        identb = singles.tile([128, 128], BF16)
        nc.vector.tensor_copy(out=identb, in_=ident32)
        one1 = singles.tile([1, 1], F32)
        nc.vector.memset(one1, 1.0)
        ones_row = singles.tile([1, 128], F32)
        nc.vector.memset(ones_row, 1.0)
        onescol = singles.tile([128, 1], BF16)
        nc.vector.memset(onescol, 1.0)

        # Load order on the SP ring: C batch 0 first (gates cast(0)),
        # then Q batch 0 (gates qprep(0)), then the other smalls. 1KB-
        # descriptor DMAs starve behind queued 8KB C loads in the DMA-
        # engine round-robin, so the smalls go before C batches 1-2.
        c_tiles = [None] * BL

        def load_c(b):
            c_t = c_pool.tile([128, NT, D], F32, name="c32")
            nc.sync.dma_start(
                out=c_t, in_=C_h[b].rearrange("(p t) d -> p t d", t=NT)
            )
            c_tiles[b] = c_t

        q_all = singles.tile([128, BL, D], F32)
        nc.sync.dma_start(
            out=q_all[:, 0:1, :],
            in_=bass.AP(tensor=Q_h, offset=0, ap=[[D, 128], [LQ * D, 1], [1, D]]),
        )
        w_row = singles.tile([1, 3 * D], F32)
        nc.sync.dma_start(
            out=w_row, in_=bass.AP(tensor=w_h, offset=0, ap=[[1, 1], [1, 3 * D]])
        )
        qm8 = singles.tile([BL, LQ], F32)
        nc.sync.dma_start(
            out=qm8, in_=bass.AP(tensor=qm_h, offset=0, ap=[[LQ, BL], [1, LQ]])
        )
        load_c(0)
        nc.sync.dma_start(
            out=q_all[:, 1:, :],
            in_=bass.AP(
                tensor=Q_h,
                offset=LQ * D,
                ap=[[D, 128], [LQ * D, BL - 1], [1, D]],
            ),
        )
        load_c(1)
        load_c(2)

        # w3T[p, k] = w[2D + 128k + p]; w2rep[p, :] = w2 broadcast
        wps = ctp_pool.tile([128, KD + D], F32, tag="ctp", name="wps")
        for k in range(KD):
            nc.tensor.matmul(
                wps[:, k : k + 1],
                w_row[:, 2 * D + 128 * k : 2 * D + 128 * (k + 1)],
                one1,
                start=True,
                stop=True,
            )
        nc.tensor.matmul(
            wps[:, KD:], ones_row, w_row[:, D : 2 * D], start=True, stop=True
        )
        w3T = singles.tile([128, KD], F32)
        nc.vector.tensor_copy(out=w3T, in_=wps[:, :KD])
        w2rep = singles.tile([128, D], F32)
        nc.vector.tensor_copy(out=w2rep, in_=wps[:, KD:])

        # qmT[j, b] = qmask[b, j] via one plain transpose-matmul
        qmT_ps = ctp_pool.tile([128, BL], F32, tag="ctp", name="qmT_ps")
        nc.tensor.matmul(qmT_ps, qm8, ident32[0:BL, 0:BL], start=True, stop=True)
        qmT = singles.tile([128, BL], F32)
        nc.vector.tensor_copy(out=qmT, in_=qmT_ps)


        # per-batch Q-side tiles. w3 is folded into the C^T copy (ct =
        # ctp * w3T rides free on the PSUM->SBUF cast), so the S matmul
        # uses plain Q^T as lhsT and no (Q*w3)^T tile exists at all.
        q_rnd = singles.tile([128, BL, D], BF16)        # Q_b bf16, rhs of U'
        qT_sb = singles.tile([128, BL, KD, 128], BF16)  # Q_b^T chunks
        bias_all = singles.tile([128, BL], F32)         # q2 - 1e4*qmask

        def qprep(b):
            """Q-side prep for batch b: q_rnd, qT, q2, bias."""
            nc.gpsimd.tensor_copy(out=q_rnd[:, b], in_=q_all[:, b])  # cast
            qT_ps = ctp_pool.tile([128, KD, 128], F32, tag="ctp", name="qT_ps")
            for k in range(KD):
                nc.tensor.matmul(
                    qT_ps[:, k],
                    q_rnd[:, b, 128 * k : 128 * (k + 1)],
                    identb,
                    start=True,
                    stop=True,
                )
            nc.vector.tensor_copy(out=qT_sb[:, b], in_=qT_ps)  # cast to bf16
            q2sb = small_pool.tile([128, 1], F32, name="q2sb")
            scr = scratch_pool.tile([128, D], F32, name="scr")
            nc.vector.scalar_tensor_tensor(
                out=scr,
                in0=q_all[:, b],
                scalar=1.0,
                in1=w2rep,
                op0=MULT,
                op1=MULT,
                accum_out=q2sb,
            )
            nc.vector.scalar_tensor_tensor(
                out=bias_all[:, b : b + 1],
                in0=qmT[:, b : b + 1],
                scalar=-10000.0,
                in1=q2sb,
                op0=MULT,
                op1=mybir.AluOpType.add,
            )

        # ---------------- per-batch pipeline stages ----------------
        def cast_c(b, h):
            """c32 half -> bf16 on ACT."""
            if h == 0:
                cast_c.cb = cb_pool.tile([128, NT, D], BF16)
            cb_t = cast_c.cb
            nc.scalar.copy(
                out=cb_t[:, 4 * h : 4 * (h + 1), :],
                in_=c_tiles[b][:, 4 * h : 4 * (h + 1), :],
            )
            return cb_t

        def stage_a(b, cb_t):
            """C^T transposes -> S^T matmul -> exp -> E (bf16)."""
            ct_t = ct_pool.tile([128, KD, LC], BF16)
            # 4 groups of 4 transposes: (half h, k-chunk k)
            for g in range(4):
                h, k = g >> 1, g & 1
                ctp = ctp_pool.tile([128, 4, 128], F32, tag="ctp")
                for tt in range(4):
                    t = 4 * h + tt
                    nc.tensor.matmul(
                        ctp[:, tt],
                        cb_t[:, t, 128 * k : 128 * (k + 1)],
                        identb,
                        start=True,
                        stop=True,
                    )
                # PSUM f32 -> SBUF bf16 copy-cast with the w3 scale folded
                # in as a per-partition scalar (3 DVE, 1 ACT)
                dst = ct_t[:, k, 512 * h : 512 * (h + 1)]
                if g == 3:
                    nc.scalar.mul(out=dst, in_=ctp, mul=w3T[:, k : k + 1])
                else:
                    nc.vector.tensor_scalar_mul(
                        out=dst, in0=ctp, scalar1=w3T[:, k : k + 1]
                    )

            e_t = e_pool.tile([128, LC], BF16)
            for h in range(2):
                s_t = s_pool.tile([128, 512], F32, tag="s")
                for k in range(KD):
                    nc.tensor.matmul(
                        s_t,
                        qT_sb[:, b, k],
                        ct_t[:, k, 512 * h : 512 * (h + 1)],
                        start=(k == 0),
                        stop=(k == KD - 1),
                    )
                nc.scalar.activation(
                    out=e_t[:, 512 * h : 512 * (h + 1)],
                    in_=s_t,
                    func=mybir.ActivationFunctionType.Exp,
                    bias=bias_all[:, b : b + 1],
                    scale=1.0,
                )
            return e_t

        def stage_b_half(b, e_t, o_t, h):
            """Half-batch epilogue: U' matmuls + denominators, one recip,
            A-scale and C*A per chunk."""
            c_t = c_tiles[b]
            u_ts = []
            sd_t = sd_pool.tile([128, 4], F32, tag="sd", name="sd_t")
            for tt in range(4):
                t = 4 * h + tt
                if tt % 2 == 0:
                    u_t = u_pool.tile([128, 2, D], F32, tag="u")
                    u_ts.append(u_t)
                e_ch = e_t[:, 128 * t : 128 * (t + 1)]
                nc.tensor.matmul(
                    u_ts[-1][:, tt % 2], e_ch, q_rnd[:, b], start=True, stop=True
                )
                nc.tensor.matmul(
                    sd_t[:, tt : tt + 1], e_ch, onescol, start=True, stop=True
                )
            r4 = small_pool.tile([128, 4], F32)
            nc.vector.reciprocal(out=r4, in_=sd_t)
            for tt in range(4):
                t = 4 * h + tt
                u_ch = u_ts[tt // 2][:, tt % 2]
                r_t = r4[:, tt : tt + 1]
                # A = U*r: 1 ACT + 3 DVE per half (PSUM read: ACT/DVE only)
                if tt == 0:
                    nc.scalar.mul(out=o_t[:, t, :D], in_=u_ch, mul=r_t)
                else:
                    nc.vector.tensor_scalar_mul(
                        out=o_t[:, t, :D], in0=u_ch, scalar1=r_t
                    )
                # C*A: 3 POOL (A*C, SBUF only) + 1 DVE fused from PSUM
                if tt < 3:
                    nc.gpsimd.tensor_mul(
                        o_t[:, t, D:], o_t[:, t, :D], c_t[:, t, :]
                    )
                else:
                    nc.vector.scalar_tensor_tensor(
                        out=o_t[:, t, D:],
                        in0=u_ch,
                        scalar=r_t,
                        in1=c_t[:, t, :],
                        op0=MULT,
                        op1=MULT,
                    )

        def store_o(b, o_t):
            """Store batch output; late batches ride the idle SP ring."""
            ring = nc.scalar if b < 5 else nc.sync
            if b == BL - 1:
                for h in range(2):
                    ring.dma_start(
                        out=bass.AP(
                            tensor=out_h,
                            offset=b * LC * 2 * D + 4 * h * 2 * D,
                            ap=[[NT * 2 * D, 128], [2 * D, 4], [1, 2 * D]],
                        ),
                        in_=o_t[:, 4 * h : 4 * (h + 1), :],
                    )
            else:
                ring.dma_start(
                    out=out_h[b].rearrange("(p t) f -> p t f", t=NT), in_=o_t
                )

        # ---------------- software-pipelined emission ----------------
        # iter b: [load(b+3); cast-h0(b+1); B(b,h0); cast-h1(b+1); B(b,h1);
        #          store(b); qprep_a(b+2); qprep_b(b+1); A(b+1)]
        qprep(0)
        cb = cast_c(0, 0)
        cast_c(0, 1)
        e_cur = stage_a(0, cb)
        qprep(1)
        for b in range(BL):
            if b + 3 < BL:
                load_c(b + 3)
            o_t = o_pool.tile([128, NT, 2 * D], F32)
            cb_nxt = cast_c(b + 1, 0) if b + 1 < BL else None
            stage_b_half(b, e_cur, o_t, 0)
            if b + 1 < BL:
                cast_c(b + 1, 1)
            stage_b_half(b, e_cur, o_t, 1)
            store_o(b, o_t)
            if b + 2 < BL:
                qprep(b + 2)
            if b + 1 < BL:
                e_cur = stage_a(b + 1, cb_nxt)
    nc.compile()
    return nc


def _get_bass() -> bass.Bass:
    if "nc" not in _CACHE:
        _CACHE["nc"] = _build_bass()
    return _CACHE["nc"]


def _run(C, Q, qmask, w, trace=False, **spmd_kwargs):
    nc = _get_bass()
    C = np.ascontiguousarray(C, dtype=np.float32)
    Q = np.ascontiguousarray(Q, dtype=np.float32)
    qmask = np.ascontiguousarray(qmask, dtype=np.float32)
    w = np.ascontiguousarray(w, dtype=np.float32)
    in_maps = [
        {
            "C": C[c * BL : (c + 1) * BL],
            "Q": Q[c * BL : (c + 1) * BL],
            "qmask": qmask[c * BL : (c + 1) * BL],
            "w": w,
        }
        for c in range(N_CORES)
    ]
    res = run_bass_kernel_spmd(
        nc, in_maps, list(range(N_CORES)), trace=trace, **spmd_kwargs
    )
    out = np.concatenate([res.results[c]["out"] for c in range(N_CORES)], axis=0)
    return out, res


def kernel(C, Q, cmask, qmask, w):
    out, _ = _run(C, Q, qmask, w, trace=False)
    return out



# revision 5
# speedup vs baseline: 1.2632x; 1.2632x over previous
"""CQAttention (QANet context-query attention) Trainium2 kernel, v6.

Full-input contract: kernel(**inputs) takes the unsharded arrays
  C [64, 1024, 256] f32, Q [64, 128, 256] f32,
  cmask [64, 1024] f32 (unused by the reference), qmask [64, 128] f32,
  w [768] f32
and returns out [64, 1024, 512] f32.

Sharding: batch dim across 8 NeuronCores (8 batches/core), no
cross-core communication.

v6 design (vs the v5 f32-I/O kernel at ~91us):
  - All big DRAM I/O in bf16 (rel-err gate is 2e-2; bf16 adds ~0.4%).
    Host casts f32->bf16 on the way in and back on the way out.
    DMA/core: 4MB CT + 0.5MB Q/QT + 8MB out = 12.6MB @ ~358GB/s ~ 35us.
  - EVERYTHING is computed transposed. The host supplies C^T [D, LC]
    and Q^T so the kernel never transposes C on-chip (v5 spent 16
    PE transposes + 2048 elem/partition of PSUM evacuation per batch).
    The output is stored transposed [2D, LC] and the host transposes
    it back (pure layout, no host FLOPs).
  - Math per batch (all on 128-partition tiles):
      S^T[j,i] = sum_d (w3*Q)^T[d,j] * C^T[d,i]   4 matmuls, k-accum
      E = exp(S^T + (q2[j] - 1e4*qmask[j]))       2 ACT ops (masked cols
                                                   underflow to 0 exactly)
      d_rep[p,i] = sum_j E[j,i]  (all-ones lhsT)  2 matmuls (replicated
                                                   across partitions to
                                                   sidestep the no-
                                                   partition-broadcast rule)
      r = 1/d_rep                                 reciprocal_approx_fast
      En = E * r                                  (softmax normalized)
      A^T[d,i] = sum_j Q[j,d] * En[j,i]           4 matmuls, lhsT = Q
                                                   as-stored (no transpose)
      out = [A^T ; C^T * A^T]                     evac copies + tensor mults
  - Engine budget/batch (measured unit costs): PE 4.3us (S 1.7 + A 1.7 +
    d 0.85), DMA 4.2us, ACT 3.4 (exp 1.4 + 3 evac copies), DVE ~4
    (recip_fast 1.3 + Enorm + 1 evac + CA), POOL ~4 (CA + q2).
  - A-phase runs one iteration behind S/D so the PE never waits on the
    recip/Enorm chain.
"""

from contextlib import ExitStack

import numpy as np
import ml_dtypes

import concourse.bacc as bacc
import concourse.bass as bass
import concourse.mybir as mybir
import concourse.tile as tile
from concourse.bass_utils import run_bass_kernel_spmd
from concourse.masks import make_identity

B, LC, LQ, D = 64, 1024, 128, 256
N_CORES = 8
BL = B // N_CORES  # batches per core
KD = D // 128      # d-chunks
F32 = mybir.dt.float32
BF16 = mybir.dt.bfloat16
MULT = mybir.AluOpType.mult
ADD = mybir.AluOpType.add
EXP = mybir.ActivationFunctionType.Exp
BF = ml_dtypes.bfloat16

_CACHE: dict = {}


def _build_bass() -> bass.Bass:
    nc = bacc.Bacc("TRN2")
    CT_h = nc.dram_tensor("CT", [BL, D, LC], BF16, kind="ExternalInput")
    Q_h = nc.dram_tensor("Qb", [BL, LQ, D], BF16, kind="ExternalInput")
    QT_h = nc.dram_tensor("QT", [KD, 128, BL, LQ], BF16, kind="ExternalInput")
    qm_h = nc.dram_tensor("qmask", [BL, LQ], F32, kind="ExternalInput")
    w_h = nc.dram_tensor("w", [3 * D], F32, kind="ExternalInput")
    out_h = nc.dram_tensor("outT", [BL, 2 * D, LC], BF16, kind="ExternalOutput")

    with tile.TileContext(nc) as tc, ExitStack() as ctx:
        singles = ctx.enter_context(tc.tile_pool(name="singles", bufs=1))
        ct_pool = ctx.enter_context(tc.tile_pool(name="ct", bufs=5))
        e_pool = ctx.enter_context(tc.tile_pool(name="e", bufs=3))
        en_pool = ctx.enter_context(tc.tile_pool(name="en", bufs=2))
        r_pool = ctx.enter_context(tc.tile_pool(name="r", bufs=2))
        oa_pool = ctx.enter_context(tc.tile_pool(name="oa", bufs=3))
        oca_pool = ctx.enter_context(tc.tile_pool(name="oca", bufs=3))
        small_pool = ctx.enter_context(tc.tile_pool(name="small", bufs=12))
        scratch_pool = ctx.enter_context(tc.tile_pool(name="scr", bufs=2))
        # PSUM: s_pool holds S^T and d_rep tiles (2 banks each, bufs=2),
        # u_pool holds A^T chunk tiles (2 banks each, bufs=2) -> 8 banks.
        s_pool = ctx.enter_context(tc.tile_pool(name="s", bufs=2, space="PSUM"))
        u_pool = ctx.enter_context(tc.tile_pool(name="u", bufs=2, space="PSUM"))

        # ---------------- one-time setup ----------------
        ident32 = singles.tile([128, 128], F32)
        make_identity(nc, ident32)
        allones = singles.tile([128, 128], BF16)
        nc.vector.memset(allones, 1.0)
        one1 = singles.tile([1, 1], F32)
        nc.vector.memset(one1, 1.0)
        ones_row = singles.tile([1, 128], F32)
        nc.vector.memset(ones_row, 1.0)

        # Small inputs first on the SP ring (they gate qprep/S of batch 0),
        # then CT batches 3 deep.
        w_row = singles.tile([1, 3 * D], F32)
        nc.sync.dma_start(
            out=w_row, in_=bass.AP(tensor=w_h, offset=0, ap=[[1, 1], [1, 3 * D]])
        )
        qm8 = singles.tile([BL, LQ], F32)
        nc.sync.dma_start(
            out=qm8, in_=bass.AP(tensor=qm_h, offset=0, ap=[[LQ, BL], [1, LQ]])
        )
        # QT packed [k, p, b, j]: per-partition 2 contiguous 2KB runs.
        qt_all = singles.tile([128, KD, BL, LQ], BF16)
        nc.sync.dma_start(
            out=qt_all,
            in_=bass.AP(
                tensor=QT_h,
                offset=0,
                ap=[[BL * LQ, 128], [128 * BL * LQ, KD], [1, BL * LQ]],
            ),
        )
        # Q as-stored [j, b, d] (lhsT of the A^T matmul).
        q_all = singles.tile([128, BL, D], BF16)
        nc.sync.dma_start(
            out=q_all,
            in_=bass.AP(tensor=Q_h, offset=0, ap=[[D, 128], [LQ * D, BL], [1, D]]),
        )

        c_tiles = [None] * BL

        def load_ct(b):
            ct_t = ct_pool.tile([128, KD, LC], BF16, name="ct")
            nc.sync.dma_start(
                out=ct_t, in_=CT_h[b].rearrange("(k p) i -> p k i", k=KD)
            )
            c_tiles[b] = ct_t

        load_ct(0)
        load_ct(1)
        load_ct(2)

        # w3T[p, k] = w[2D + 128k + p]; w2rep[p, :] = w2 broadcast down
        # partitions (via PE outer products, evac'd once).
        wps = s_pool.tile([128, KD + D], F32, name="wps", tag="s")
        for k in range(KD):
            nc.tensor.matmul(
                wps[:, k : k + 1],
                w_row[:, 2 * D + 128 * k : 2 * D + 128 * (k + 1)],
                one1,
                start=True,
                stop=True,
            )
        nc.tensor.matmul(
            wps[:, KD:], ones_row, w_row[:, D : 2 * D], start=True, stop=True
        )
        w3T = singles.tile([128, KD], F32)
        nc.vector.tensor_copy(out=w3T, in_=wps[:, :KD])
        w2rep = singles.tile([128, D], F32)
        nc.vector.tensor_copy(out=w2rep, in_=wps[:, KD:])

        # qmT[j, b] = qmask[b, j]
        qmT_ps = u_pool.tile([128, BL], F32, name="qmT_ps", tag="u")
        nc.tensor.matmul(qmT_ps, qm8, ident32[0:BL, 0:BL], start=True, stop=True)
        qmT = singles.tile([128, BL], F32)
        nc.vector.tensor_copy(out=qmT, in_=qmT_ps)

        # qtw3[p, k, b, j] = w3[128k+p] * Q^T[128k+p, b, j]  (one-time)
        qtw3 = singles.tile([128, KD, BL, LQ], BF16)
        for k in range(KD):
            nc.vector.tensor_scalar_mul(
                out=qtw3[:, k], in0=qt_all[:, k], scalar1=w3T[:, k : k + 1]
            )

        bias_all = singles.tile([128, BL], F32)

        def qprep(b):
            """bias[:, b] = q2 - 1e4*qmask for batch b (POOL + DVE smalls)."""
            q2sb = small_pool.tile([128, 1], F32, name="q2sb")
            scr = scratch_pool.tile([128, D], F32, name="scr")
            nc.vector.scalar_tensor_tensor(
                out=scr,
                in0=q_all[:, b],
                scalar=1.0,
                in1=w2rep,
                op0=MULT,
                op1=MULT,
                accum_out=q2sb,
            )
            nc.vector.scalar_tensor_tensor(
                out=bias_all[:, b : b + 1],
                in0=qmT[:, b : b + 1],
                scalar=-10000.0,
                in1=q2sb,
                op0=MULT,
                op1=ADD,
            )

        # ---------------- per-batch pipeline stages ----------------
        def stage_s(b):
            """S^T matmuls + exp -> E [j, i] bf16."""
            ct_t = c_tiles[b]
            s_t = s_pool.tile([128, 2, 512], F32, name="s_t", tag="s")
            for h in range(2):
                for k in range(KD):
                    nc.tensor.matmul(
                        s_t[:, h],
                        qtw3[:, k, b],
                        ct_t[:, k, 512 * h : 512 * (h + 1)],
                        start=(k == 0),
                        stop=(k == KD - 1),
                    )
            e_t = e_pool.tile([128, LC], BF16)
            for h in range(2):
                nc.scalar.activation(
                    out=e_t[:, 512 * h : 512 * (h + 1)],
                    in_=s_t[:, h],
                    func=EXP,
                    bias=bias_all[:, b : b + 1],
                    scale=1.0,
                )
            return e_t

        def stage_d(b, e_t):
            """Softmax denominator, replicated across partitions, then
            En = E / d via reciprocal_approx_fast + multiply."""
            d_ps = s_pool.tile([128, 2, 512], F32, name="d_ps", tag="s")
            for h in range(2):
                nc.tensor.matmul(
                    d_ps[:, h],
                    allones,
                    e_t[:, 512 * h : 512 * (h + 1)],
                    start=True,
                    stop=True,
                )
            r_t = r_pool.tile([128, 2, 512], F32)
            nc.vector.reciprocal_approx_fast(out=r_t, in_=d_ps)
            en_t = en_pool.tile([128, LC], BF16)
            for h in range(2):
                sl = slice(512 * h, 512 * (h + 1))
                eng = nc.vector if h == 0 else nc.gpsimd
                eng.tensor_tensor(
                    out=en_t[:, sl], in0=e_t[:, sl], in1=r_t[:, h], op=MULT
                )
            return en_t

        def stage_a(b, en_t):
            """A^T matmuls (lhsT = Q as stored), evac + C^T*A^T, store."""
            ct_t = c_tiles[b]
            o_a = oa_pool.tile([128, KD, LC], BF16)
            o_ca = oca_pool.tile([128, KD, LC], BF16)
            for k in range(KD):
                u_t = u_pool.tile([128, 2, 512], F32, name="u_t", tag="u")
                for h in range(2):
                    nc.tensor.matmul(
                        u_t[:, h],
                        q_all[:, b, 128 * k : 128 * (k + 1)],
                        en_t[:, 512 * h : 512 * (h + 1)],
                        start=True,
                        stop=True,
                    )
                for h in range(2):
                    sl = slice(512 * h, 512 * (h + 1))
                    # evac: 3 ACT copies + 1 DVE
                    if k == 1 and h == 1:
                        nc.vector.tensor_copy(out=o_a[:, k, sl], in_=u_t[:, h])
                    else:
                        nc.scalar.copy(out=o_a[:, k, sl], in_=u_t[:, h])
                    # CA: 3 POOL + 1 DVE (SBUF bf16 x bf16)
                    if k == 0 and h == 0:
                        nc.vector.tensor_tensor(
                            out=o_ca[:, k, sl], in0=o_a[:, k, sl],
                            in1=ct_t[:, k, sl], op=MULT,
                        )
                    else:
                        nc.gpsimd.tensor_tensor(
                            out=o_ca[:, k, sl], in0=o_a[:, k, sl],
                            in1=ct_t[:, k, sl], op=MULT,
                        )
            return o_a, o_ca

        def store_o(b, o_a, o_ca):
            base = b * 2 * D * LC
            nc.scalar.dma_start(
                out=bass.AP(
                    tensor=out_h,
                    offset=base,
                    ap=[[LC, 128], [128 * LC, KD], [1, LC]],
                ),
                in_=o_a,
            )
            nc.sync.dma_start(
                out=bass.AP(
                    tensor=out_h,
                    offset=base + D * LC,
                    ap=[[LC, 128], [128 * LC, KD], [1, LC]],
                ),
                in_=o_ca,
            )

        # ---------------- software-pipelined emission ----------------
        # A-phase runs one iteration behind S/D so the PE never stalls on
        # the recip/Enorm chain.
        qprep(0)
        qprep(1)
        e_cur = stage_s(0)
        en_tiles = {0: stage_d(0, e_cur)}
        e_tiles = {0: e_cur}
        for b in range(BL + 1):
            if b + 3 < BL:
                load_ct(b + 3)
            if b >= 1:
                o_a, o_ca = stage_a(b - 1, en_tiles.pop(b - 1))
            if b + 1 < BL:
                e_tiles[b + 1] = stage_s(b + 1)
            if b >= 1:
                store_o(b - 1, o_a, o_ca)
            if b < BL:
                if b >= 1:
                    en_tiles[b] = stage_d(b, e_tiles.pop(b))
                if b + 2 < BL:
                    qprep(b + 2)
    nc.compile()
    return nc


def _get_bass() -> bass.Bass:
    if "nc" not in _CACHE:
        _CACHE["nc"] = _build_bass()
    return _CACHE["nc"]


def _prep_core_inputs(C, Q, qmask, w, c):
    sl = slice(c * BL, (c + 1) * BL)
    Cc = C[sl]                                   # [BL, LC, D] f32
    CT = Cc.transpose(0, 2, 1).astype(BF)        # [BL, D, LC] bf16
    Qc = Q[sl].astype(BF)                        # [BL, LQ, D] bf16
    QT = np.ascontiguousarray(Qc.transpose(2, 0, 1)).reshape(KD, 128, BL, LQ)
    return {
        "CT": CT,
        "Qb": Qc,
        "QT": QT,
        "qmask": np.ascontiguousarray(qmask[sl], dtype=np.float32),
        "w": np.ascontiguousarray(w, dtype=np.float32),
    }


def _run(C, Q, qmask, w, trace=False, **spmd_kwargs):
    nc = _get_bass()
    C = np.ascontiguousarray(C, dtype=np.float32)
    Q = np.ascontiguousarray(Q, dtype=np.float32)
    qmask = np.ascontiguousarray(qmask, dtype=np.float32)
    w = np.ascontiguousarray(w, dtype=np.float32)
    in_maps = [_prep_core_inputs(C, Q, qmask, w, c) for c in range(N_CORES)]
    res = run_bass_kernel_spmd(
        nc, in_maps, list(range(N_CORES)), trace=trace, **spmd_kwargs
    )
    outT = np.concatenate(
        [np.asarray(res.results[c]["outT"]) for c in range(N_CORES)], axis=0
    )  # [B, 2D, LC] bf16
    out = np.ascontiguousarray(outT.astype(np.float32).transpose(0, 2, 1))
    return out, res


def kernel(C, Q, cmask, qmask, w):
    out, _ = _run(C, Q, qmask, w, trace=False)
    return out


# revision 7
# speedup vs baseline: 1.2836x; 1.0161x over previous
"""CQAttention (QANet context-query attention) Trainium2 kernel, v6.1.

Full-input contract: kernel(**inputs) takes the unsharded arrays
  C [64, 1024, 256] f32, Q [64, 128, 256] f32,
  cmask [64, 1024] f32 (unused by the reference), qmask [64, 128] f32,
  w [768] f32
and returns out [64, 1024, 512] f32.

Sharding: batch dim across 8 NeuronCores (8 batches/core), no
cross-core communication.

Design (history: v5 f32-I/O 91us, v6.0 73.6us):
  - All big DRAM I/O in bf16 (rel-err gate 2e-2; bf16 adds ~0.4%).
  - Everything computed transposed: host supplies C^T and Q^T packed so
    the kernel never transposes on-chip; output leaves as [2D, LC]^T and
    the host transposes back (layout only, no host FLOPs).
  - DMA queues are packet-rate-limited (~170ns/packet/DGE-engine
    measured), so host packs arrays to give 4KB (loads) / 8KB (store)
    contiguous runs per partition, and the whole batch output goes out
    in ONE dma_start.
  - Per batch: S^T = (w3*Q)^T @ C^T (4 mm) -> exp+bias (2 ACT) ->
    d = ones@E replicated across partitions (2 mm) ->
    r = reciprocal_approx_fast (1 DVE) -> En = E*r (DVE+POOL) ->
    A^T = Q @ En (4 mm, normalized) -> evac copies (4 ACT) +
    CA = u*C^T from PSUM (3 DVE) / from SBUF (1 POOL).
  - A-phase lags one iteration so the PE never waits on recip/Enorm.
    PE order per iter: d(b), S(b+1), A(b-1).
"""

from contextlib import ExitStack

import numpy as np
import ml_dtypes

import concourse.bacc as bacc
import concourse.bass as bass
import concourse.mybir as mybir
import concourse.tile as tile
from concourse.bass_utils import run_bass_kernel_spmd
from concourse.masks import make_identity

B, LC, LQ, D = 64, 1024, 128, 256
N_CORES = 8
BL = B // N_CORES  # batches per core
KD = D // 128      # d-chunks
F32 = mybir.dt.float32
BF16 = mybir.dt.bfloat16
MULT = mybir.AluOpType.mult
ADD = mybir.AluOpType.add
EXP = mybir.ActivationFunctionType.Exp
BF = ml_dtypes.bfloat16

_CACHE: dict = {}


def _build_bass() -> bass.Bass:
    nc = bacc.Bacc("TRN2")
    # CTC[b, p, k, i] = C[b, i, 128k+p] * nothing (raw C^T, packed so each
    # partition reads 4KB contiguous).
    CT_h = nc.dram_tensor("CTC", [BL, 128, KD, LC], BF16, kind="ExternalInput")
    Q_h = nc.dram_tensor("Qb", [BL, LQ, D], BF16, kind="ExternalInput")
    QT_h = nc.dram_tensor("QT", [KD, 128, BL, LQ], BF16, kind="ExternalInput")
    qm_h = nc.dram_tensor("qmask", [BL, LQ], F32, kind="ExternalInput")
    w_h = nc.dram_tensor("w", [3 * D], F32, kind="ExternalInput")
    # OTP[b, p, g, i]: g in {A k0, A k1, CA k0, CA k1}; 8KB/partition runs.
    out_h = nc.dram_tensor("outT", [BL, 128, 4, LC], BF16, kind="ExternalOutput")

    with tile.TileContext(nc) as tc, ExitStack() as ctx:
        singles = ctx.enter_context(tc.tile_pool(name="singles", bufs=1))
        ct_pool = ctx.enter_context(tc.tile_pool(name="ct", bufs=5))
        e_pool = ctx.enter_context(tc.tile_pool(name="e", bufs=3))
        en_pool = ctx.enter_context(tc.tile_pool(name="en", bufs=2))
        r_pool = ctx.enter_context(tc.tile_pool(name="r", bufs=2))
        o_pool = ctx.enter_context(tc.tile_pool(name="o", bufs=3))
        small_pool = ctx.enter_context(tc.tile_pool(name="small", bufs=12))
        scratch_pool = ctx.enter_context(tc.tile_pool(name="scr", bufs=2))
        s_pool = ctx.enter_context(tc.tile_pool(name="s", bufs=2, space="PSUM"))
        u_pool = ctx.enter_context(tc.tile_pool(name="u", bufs=2, space="PSUM"))

        # ---------------- one-time setup ----------------
        ident32 = singles.tile([128, 128], F32)
        make_identity(nc, ident32)
        allones = singles.tile([128, 128], BF16)
        nc.vector.memset(allones, 1.0)
        one1 = singles.tile([1, 1], F32)
        nc.vector.memset(one1, 1.0)
        ones_row = singles.tile([1, 128], F32)
        nc.vector.memset(ones_row, 1.0)

        w_row = singles.tile([1, 3 * D], F32)
        nc.sync.dma_start(
            out=w_row, in_=bass.AP(tensor=w_h, offset=0, ap=[[1, 1], [1, 3 * D]])
        )
        qm8 = singles.tile([BL, LQ], F32)
        nc.sync.dma_start(
            out=qm8, in_=bass.AP(tensor=qm_h, offset=0, ap=[[LQ, BL], [1, LQ]])
        )
        qt_all = singles.tile([128, KD, BL, LQ], BF16)
        nc.sync.dma_start(
            out=qt_all,
            in_=bass.AP(
                tensor=QT_h,
                offset=0,
                ap=[[BL * LQ, 128], [128 * BL * LQ, KD], [1, BL * LQ]],
            ),
        )
        q_all = singles.tile([128, BL, D], BF16)
        nc.sync.dma_start(
            out=q_all,
            in_=bass.AP(tensor=Q_h, offset=0, ap=[[D, 128], [LQ * D, BL], [1, D]]),
        )

        c_tiles = [None] * BL

        def load_ct(b):
            ct_t = ct_pool.tile([128, KD, LC], BF16, name="ct")
            nc.sync.dma_start(out=ct_t, in_=CT_h[b])
            c_tiles[b] = ct_t

        load_ct(0)
        load_ct(1)
        load_ct(2)

        # w3T[p, k] = w[2D + 128k + p]; w2rep[p, :] = w2 broadcast.
        wps = s_pool.tile([128, KD + D], F32, name="wps", tag="s")
        for k in range(KD):
            nc.tensor.matmul(
                wps[:, k : k + 1],
                w_row[:, 2 * D + 128 * k : 2 * D + 128 * (k + 1)],
                one1,
                start=True,
                stop=True,
            )
        nc.tensor.matmul(
            wps[:, KD:], ones_row, w_row[:, D : 2 * D], start=True, stop=True
        )
        w3T = singles.tile([128, KD], F32)
        nc.vector.tensor_copy(out=w3T, in_=wps[:, :KD])
        w2rep = singles.tile([128, D], F32)
        nc.vector.tensor_copy(out=w2rep, in_=wps[:, KD:])

        qmT_ps = u_pool.tile([128, BL], F32, name="qmT_ps", tag="u")
        nc.tensor.matmul(qmT_ps, qm8, ident32[0:BL, 0:BL], start=True, stop=True)
        qmT = singles.tile([128, BL], F32)
        nc.vector.tensor_copy(out=qmT, in_=qmT_ps)

        # qtw3[p, k, b, j] = w3[128k+p] * Q^T[128k+p, b, j].
        # Batch 0 first (tiny, unblocks S(0)); batches 1.. in one bulk op
        # per k while the head of the pipeline is otherwise idle on DVE.
        qtw3 = singles.tile([128, KD, BL, LQ], BF16)

        def qtw3_prep(b_slice, eng):
            for k in range(KD):
                eng.tensor_scalar_mul(
                    out=qtw3[:, k, b_slice],
                    in0=qt_all[:, k, b_slice],
                    scalar1=w3T[:, k : k + 1],
                )

        bias_all = singles.tile([128, BL], F32)

        def qprep(b):
            q2sb = small_pool.tile([128, 1], F32, name="q2sb")
            scr = scratch_pool.tile([128, D], F32, name="scr")
            nc.vector.scalar_tensor_tensor(
                out=scr,
                in0=q_all[:, b],
                scalar=1.0,
                in1=w2rep,
                op0=MULT,
                op1=MULT,
                accum_out=q2sb,
            )
            nc.vector.scalar_tensor_tensor(
                out=bias_all[:, b : b + 1],
                in0=qmT[:, b : b + 1],
                scalar=-10000.0,
                in1=q2sb,
                op0=MULT,
                op1=ADD,
            )

        # ---------------- per-batch pipeline stages ----------------
        def stage_s(b):
            ct_t = c_tiles[b]
            s_t = s_pool.tile([128, 2, 512], F32, name="s_t", tag="s")
            for h in range(2):
                for k in range(KD):
                    nc.tensor.matmul(
                        s_t[:, h],
                        qtw3[:, k, b],
                        ct_t[:, k, 512 * h : 512 * (h + 1)],
                        start=(k == 0),
                        stop=(k == KD - 1),
                    )
            e_t = e_pool.tile([128, LC], BF16)
            for h in range(2):
                nc.scalar.activation(
                    out=e_t[:, 512 * h : 512 * (h + 1)],
                    in_=s_t[:, h],
                    func=EXP,
                    bias=bias_all[:, b : b + 1],
                    scale=1.0,
                )
            return e_t

        def stage_d_mm(b, e_t):
            d_ps = s_pool.tile([128, 2, 512], F32, name="d_ps", tag="s")
            for h in range(2):
                nc.tensor.matmul(
                    d_ps[:, h],
                    allones,
                    e_t[:, 512 * h : 512 * (h + 1)],
                    start=True,
                    stop=True,
                )
            return d_ps

        def stage_d_vec(b, e_t, d_ps):
            r_t = r_pool.tile([128, 2, 512], F32)
            nc.vector.reciprocal_approx_fast(out=r_t, in_=d_ps)
            en_t = en_pool.tile([128, LC], BF16)
            # Enorm: both halves on POOL (they're ready early; POOL's only
            # other work also has early inputs).
            for h in range(2):
                sl = slice(512 * h, 512 * (h + 1))
                nc.gpsimd.tensor_tensor(
                    out=en_t[:, sl], in0=e_t[:, sl], in1=r_t[:, h], op=MULT
                )
            return en_t

        def stage_a(b, en_t):
            """A^T matmuls; evac on ACT; CA mostly from PSUM on DVE."""
            ct_t = c_tiles[b]
            o_t = o_pool.tile([128, 4, LC], BF16)
            for k in range(KD):
                u_t = u_pool.tile([128, 2, 512], F32, name="u_t", tag="u")
                for h in range(2):
                    nc.tensor.matmul(
                        u_t[:, h],
                        q_all[:, b, 128 * k : 128 * (k + 1)],
                        en_t[:, 512 * h : 512 * (h + 1)],
                        start=True,
                        stop=True,
                    )
                for h in range(2):
                    sl = slice(512 * h, 512 * (h + 1))
                    nc.scalar.copy(out=o_t[:, k, sl], in_=u_t[:, h])
                    # CA: u is already normalized; 3 chunks on DVE straight
                    # from PSUM, 1 on POOL from the evac'd SBUF copy.
                    if k == 0 and h == 0:
                        nc.gpsimd.tensor_tensor(
                            out=o_t[:, 2 + k, sl], in0=o_t[:, k, sl],
                            in1=ct_t[:, k, sl], op=MULT,
                        )
                    else:
                        nc.vector.tensor_tensor(
                            out=o_t[:, 2 + k, sl], in0=u_t[:, h],
                            in1=ct_t[:, k, sl], op=MULT,
                        )
            return o_t

        def store_o(b, o_t):
            nc.sync.dma_start(out=out_h[b], in_=o_t)

        # ---------------- software-pipelined emission ----------------
        qprep(0)
        qtw3_prep(slice(0, 1), nc.vector)
        qprep(1)
        e_tiles = {0: stage_s(0)}
        qtw3_prep(slice(1, BL), nc.vector)  # bulk, off the critical path
        d_cur = stage_d_mm(0, e_tiles[0])
        en_tiles = {0: stage_d_vec(0, e_tiles[0], d_cur)}
        pending_store = None
        for b in range(BL + 1):
            if b + 3 < BL:
                load_ct(b + 3)
            if pending_store is not None:
                store_o(*pending_store)
                pending_store = None
            # PE order: d(b) first (inputs long ready), then S(b+1), then
            # A(b-1); recip/Enorm(b) start right after d(b).
            if 1 <= b < BL:
                d_ps = stage_d_mm(b, e_tiles[b])
                r_en = (b, e_tiles.pop(b), d_ps)
            else:
                r_en = None
            if b + 1 < BL:
                e_tiles[b + 1] = stage_s(b + 1)
            if r_en is not None:
                en_tiles[b] = stage_d_vec(*r_en)
            if b >= 1:
                pending_store = (b - 1, stage_a(b - 1, en_tiles.pop(b - 1)))
            if b + 2 < BL:
                qprep(b + 2)
        store_o(*pending_store)
    nc.compile()
    return nc


def _get_bass() -> bass.Bass:
    if "nc" not in _CACHE:
        _CACHE["nc"] = _build_bass()
    return _CACHE["nc"]


def _prep_core_inputs(C, Q, qmask, w, c):
    sl = slice(c * BL, (c + 1) * BL)
    Cc = C[sl]                                   # [BL, LC, D] f32
    # CTC[b, p, k, i] = C[b, i, 128k+p]
    CT = Cc.transpose(0, 2, 1).astype(BF)        # [BL, D, LC]
    CTC = np.ascontiguousarray(
        CT.reshape(BL, KD, 128, LC).transpose(0, 2, 1, 3)
    )                                            # [BL, 128, KD, LC]
    Qc = Q[sl].astype(BF)                        # [BL, LQ, D] bf16
    QT = np.ascontiguousarray(Qc.transpose(2, 0, 1)).reshape(KD, 128, BL, LQ)
    return {
        "CTC": CTC,
        "Qb": Qc,
        "QT": QT,
        "qmask": np.ascontiguousarray(qmask[sl], dtype=np.float32),
        "w": np.ascontiguousarray(w, dtype=np.float32),
    }


def _run(C, Q, qmask, w, trace=False, **spmd_kwargs):
    nc = _get_bass()
    C = np.ascontiguousarray(C, dtype=np.float32)
    Q = np.ascontiguousarray(Q, dtype=np.float32)
    qmask = np.ascontiguousarray(qmask, dtype=np.float32)
    w = np.ascontiguousarray(w, dtype=np.float32)
    in_maps = [_prep_core_inputs(C, Q, qmask, w, c) for c in range(N_CORES)]
    res = run_bass_kernel_spmd(
        nc, in_maps, list(range(N_CORES)), trace=trace, **spmd_kwargs
    )
    outT = np.concatenate(
        [np.asarray(res.results[c]["outT"]) for c in range(N_CORES)], axis=0
    )  # [B, 128, 4, LC] bf16: [b, p, (a k | ca k), i]
    # out[b, i, 128k+p (+256 for ca)] = outT[b, p, g, i]
    out = np.ascontiguousarray(
        outT.astype(np.float32).transpose(0, 3, 2, 1).reshape(B, LC, 2 * D)
    )
    return out, res


def kernel(C, Q, cmask, qmask, w):
    out, _ = _run(C, Q, qmask, w, trace=False)
    return out


# revision 8
# speedup vs baseline: 1.3163x; 1.0255x over previous
"""CQAttention (QANet context-query attention) Trainium2 kernel, v6.1.

Full-input contract: kernel(**inputs) takes the unsharded arrays
  C [64, 1024, 256] f32, Q [64, 128, 256] f32,
  cmask [64, 1024] f32 (unused by the reference), qmask [64, 128] f32,
  w [768] f32
and returns out [64, 1024, 512] f32.

Sharding: batch dim across 8 NeuronCores (8 batches/core), no
cross-core communication.

Design (history: v5 f32-I/O 91us, v6.0 73.6us):
  - All big DRAM I/O in bf16 (rel-err gate 2e-2; bf16 adds ~0.4%).
  - Everything computed transposed: host supplies C^T and Q^T packed so
    the kernel never transposes on-chip; output leaves as [2D, LC]^T and
    the host transposes back (layout only, no host FLOPs).
  - DMA queues are packet-rate-limited (~170ns/packet/DGE-engine
    measured), so host packs arrays to give 4KB (loads) / 8KB (store)
    contiguous runs per partition, and the whole batch output goes out
    in ONE dma_start.
  - Per batch: S^T = (w3*Q)^T @ C^T (4 mm) -> exp+bias (2 ACT) ->
    d = ones@E replicated across partitions (2 mm) ->
    r = reciprocal_approx_fast (1 DVE) -> En = E*r (DVE+POOL) ->
    A^T = Q @ En (4 mm, normalized) -> evac copies (4 ACT) +
    CA = u*C^T from PSUM (3 DVE) / from SBUF (1 POOL).
  - A-phase lags one iteration so the PE never waits on recip/Enorm.
    PE order per iter: d(b), S(b+1), A(b-1).
"""

from contextlib import ExitStack

import numpy as np
import ml_dtypes

import concourse.bacc as bacc
import concourse.bass as bass
import concourse.mybir as mybir
import concourse.tile as tile
from concourse.bass_utils import run_bass_kernel_spmd
from concourse.masks import make_identity

B, LC, LQ, D = 64, 1024, 128, 256
N_CORES = 8
BL = B // N_CORES  # batches per core
KD = D // 128      # d-chunks
F32 = mybir.dt.float32
BF16 = mybir.dt.bfloat16
MULT = mybir.AluOpType.mult
ADD = mybir.AluOpType.add
EXP = mybir.ActivationFunctionType.Exp
BF = ml_dtypes.bfloat16

_CACHE: dict = {}


def _build_bass() -> bass.Bass:
    nc = bacc.Bacc("TRN2")
    # CTC[b, p, k, i] = C[b, i, 128k+p] * nothing (raw C^T, packed so each
    # partition reads 4KB contiguous).
    CT_h = nc.dram_tensor("CTC", [BL, 128, KD, LC], BF16, kind="ExternalInput")
    Q_h = nc.dram_tensor("Qb", [LQ, BL, D], BF16, kind="ExternalInput")
    QT_h = nc.dram_tensor("QT", [128, KD, BL, LQ], BF16, kind="ExternalInput")
    qm_h = nc.dram_tensor("qmask", [BL, LQ], F32, kind="ExternalInput")
    w_h = nc.dram_tensor("w", [3 * D], F32, kind="ExternalInput")
    # OTP[b, p, g, i]: g in {A k0, A k1, CA k0, CA k1}; 8KB/partition runs.
    out_h = nc.dram_tensor("outT", [BL, 128, 4, LC], BF16, kind="ExternalOutput")

    with tile.TileContext(nc) as tc, ExitStack() as ctx:
        singles = ctx.enter_context(tc.tile_pool(name="singles", bufs=1))
        ct_pool = ctx.enter_context(tc.tile_pool(name="ct", bufs=5))
        e_pool = ctx.enter_context(tc.tile_pool(name="e", bufs=3))
        en_pool = ctx.enter_context(tc.tile_pool(name="en", bufs=2))
        r_pool = ctx.enter_context(tc.tile_pool(name="r", bufs=2))
        o_pool = ctx.enter_context(tc.tile_pool(name="o", bufs=3))
        small_pool = ctx.enter_context(tc.tile_pool(name="small", bufs=12))
        scratch_pool = ctx.enter_context(tc.tile_pool(name="scr", bufs=2))
        s_pool = ctx.enter_context(tc.tile_pool(name="s", bufs=2, space="PSUM"))
        u_pool = ctx.enter_context(tc.tile_pool(name="u", bufs=2, space="PSUM"))

        # ---------------- one-time setup ----------------
        ident32 = singles.tile([128, 128], F32)
        make_identity(nc, ident32)
        allones = singles.tile([128, 128], BF16)
        nc.vector.memset(allones, 1.0)
        one1 = singles.tile([1, 1], F32)
        nc.vector.memset(one1, 1.0)
        ones_row = singles.tile([1, 128], F32)
        nc.vector.memset(ones_row, 1.0)

        w_row = singles.tile([1, 3 * D], F32)
        nc.sync.dma_start(
            out=w_row, in_=bass.AP(tensor=w_h, offset=0, ap=[[1, 1], [1, 3 * D]])
        )
        qm8 = singles.tile([BL, LQ], F32)
        nc.sync.dma_start(
            out=qm8, in_=bass.AP(tensor=qm_h, offset=0, ap=[[LQ, BL], [1, LQ]])
        )
        qt_all = singles.tile([128, KD, BL, LQ], BF16)
        nc.sync.dma_start(
            out=qt_all,
            in_=bass.AP(
                tensor=QT_h,
                offset=0,
                ap=[[KD * BL * LQ, 128], [BL * LQ, KD], [1, BL * LQ]],
            ),
        )
        q_all = singles.tile([128, BL, D], BF16)
        nc.sync.dma_start(
            out=q_all,
            in_=bass.AP(
                tensor=Q_h, offset=0, ap=[[BL * D, 128], [D, BL], [1, D]]
            ),
        )

        c_tiles = [None] * BL

        def load_ct(b):
            ct_t = ct_pool.tile([128, KD, LC], BF16, name="ct")
            nc.sync.dma_start(out=ct_t, in_=CT_h[b])
            c_tiles[b] = ct_t

        load_ct(0)
        load_ct(1)
        load_ct(2)

        # w3T[p, k] = w[2D + 128k + p]; w2rep[p, :] = w2 broadcast.
        wps = s_pool.tile([128, KD + D], F32, name="wps", tag="s")
        for k in range(KD):
            nc.tensor.matmul(
                wps[:, k : k + 1],
                w_row[:, 2 * D + 128 * k : 2 * D + 128 * (k + 1)],
                one1,
                start=True,
                stop=True,
            )
        nc.tensor.matmul(
            wps[:, KD:], ones_row, w_row[:, D : 2 * D], start=True, stop=True
        )
        w3T = singles.tile([128, KD], F32)
        nc.vector.tensor_copy(out=w3T, in_=wps[:, :KD])
        w2rep = singles.tile([128, D], F32)
        nc.vector.tensor_copy(out=w2rep, in_=wps[:, KD:])

        qmT_ps = u_pool.tile([128, BL], F32, name="qmT_ps", tag="u")
        nc.tensor.matmul(qmT_ps, qm8, ident32[0:BL, 0:BL], start=True, stop=True)
        qmT = singles.tile([128, BL], F32)
        nc.vector.tensor_copy(out=qmT, in_=qmT_ps)

        # qtw3[p, k, b, j] = w3[128k+p] * Q^T[128k+p, b, j].
        # Batch 0 first (tiny, unblocks S(0)); batches 1.. in one bulk op
        # per k while the head of the pipeline is otherwise idle on DVE.
        qtw3 = singles.tile([128, KD, BL, LQ], BF16)

        def qtw3_prep(b_slice, eng):
            for k in range(KD):
                eng.tensor_scalar_mul(
                    out=qtw3[:, k, b_slice],
                    in0=qt_all[:, k, b_slice],
                    scalar1=w3T[:, k : k + 1],
                )

        bias_all = singles.tile([128, BL], F32)

        def qprep(b):
            q2sb = small_pool.tile([128, 1], F32, name="q2sb")
            scr = scratch_pool.tile([128, D], F32, name="scr")
            nc.vector.scalar_tensor_tensor(
                out=scr,
                in0=q_all[:, b],
                scalar=1.0,
                in1=w2rep,
                op0=MULT,
                op1=MULT,
                accum_out=q2sb,
            )
            nc.vector.scalar_tensor_tensor(
                out=bias_all[:, b : b + 1],
                in0=qmT[:, b : b + 1],
                scalar=-10000.0,
                in1=q2sb,
                op0=MULT,
                op1=ADD,
            )

        # ---------------- per-batch pipeline stages ----------------
        def stage_s(b):
            ct_t = c_tiles[b]
            s_t = s_pool.tile([128, 2, 512], F32, name="s_t", tag="s")
            for h in range(2):
                for k in range(KD):
                    nc.tensor.matmul(
                        s_t[:, h],
                        qtw3[:, k, b],
                        ct_t[:, k, 512 * h : 512 * (h + 1)],
                        start=(k == 0),
                        stop=(k == KD - 1),
                    )
            e_t = e_pool.tile([128, LC], BF16)
            for h in range(2):
                nc.scalar.activation(
                    out=e_t[:, 512 * h : 512 * (h + 1)],
                    in_=s_t[:, h],
                    func=EXP,
                    bias=bias_all[:, b : b + 1],
                    scale=1.0,
                )
            return e_t

        def stage_d_mm(b, e_t):
            d_ps = s_pool.tile([128, 2, 512], F32, name="d_ps", tag="s")
            for h in range(2):
                nc.tensor.matmul(
                    d_ps[:, h],
                    allones,
                    e_t[:, 512 * h : 512 * (h + 1)],
                    start=True,
                    stop=True,
                )
            return d_ps

        def stage_d_vec(b, e_t, d_ps):
            r_t = r_pool.tile([128, 2, 512], F32)
            nc.vector.reciprocal_approx_fast(out=r_t, in_=d_ps)
            en_t = en_pool.tile([128, LC], BF16)
            # Enorm: both halves on POOL (they're ready early; POOL's only
            # other work also has early inputs).
            for h in range(2):
                sl = slice(512 * h, 512 * (h + 1))
                nc.gpsimd.tensor_tensor(
                    out=en_t[:, sl], in0=e_t[:, sl], in1=r_t[:, h], op=MULT
                )
            return en_t

        def stage_a(b, en_t):
            """A^T matmuls; evac on ACT; CA mostly from PSUM on DVE."""
            ct_t = c_tiles[b]
            o_t = o_pool.tile([128, 4, LC], BF16)
            for k in range(KD):
                u_t = u_pool.tile([128, 2, 512], F32, name="u_t", tag="u")
                for h in range(2):
                    nc.tensor.matmul(
                        u_t[:, h],
                        q_all[:, b, 128 * k : 128 * (k + 1)],
                        en_t[:, 512 * h : 512 * (h + 1)],
                        start=True,
                        stop=True,
                    )
                for h in range(2):
                    sl = slice(512 * h, 512 * (h + 1))
                    nc.scalar.copy(out=o_t[:, k, sl], in_=u_t[:, h])
                    # CA: u is already normalized; 3 chunks on DVE straight
                    # from PSUM, 1 on POOL from the evac'd SBUF copy.
                    if k == 0 and h == 0:
                        nc.gpsimd.tensor_tensor(
                            out=o_t[:, 2 + k, sl], in0=o_t[:, k, sl],
                            in1=ct_t[:, k, sl], op=MULT,
                        )
                    else:
                        nc.vector.tensor_tensor(
                            out=o_t[:, 2 + k, sl], in0=u_t[:, h],
                            in1=ct_t[:, k, sl], op=MULT,
                        )
            return o_t

        def store_o(b, o_t):
            nc.sync.dma_start(out=out_h[b], in_=o_t)

        # ---------------- software-pipelined emission ----------------
        qprep(0)
        qtw3_prep(slice(0, 1), nc.vector)
        qprep(1)
        e_tiles = {0: stage_s(0)}
        qtw3_prep(slice(1, BL), nc.vector)  # bulk, off the critical path
        d_cur = stage_d_mm(0, e_tiles[0])
        en_tiles = {0: stage_d_vec(0, e_tiles[0], d_cur)}
        pending_store = None
        for b in range(BL + 1):
            if b + 3 < BL:
                load_ct(b + 3)
            if pending_store is not None:
                store_o(*pending_store)
                pending_store = None
            # PE order: d(b) first (inputs long ready), then S(b+1), then
            # A(b-1); recip/Enorm(b) start right after d(b).
            if 1 <= b < BL:
                d_ps = stage_d_mm(b, e_tiles[b])
                r_en = (b, e_tiles.pop(b), d_ps)
            else:
                r_en = None
            if b + 1 < BL:
                e_tiles[b + 1] = stage_s(b + 1)
            if r_en is not None:
                en_tiles[b] = stage_d_vec(*r_en)
            if b >= 1:
                pending_store = (b - 1, stage_a(b - 1, en_tiles.pop(b - 1)))
            if b + 2 < BL:
                qprep(b + 2)
        store_o(*pending_store)
    nc.compile()
    return nc


def _get_bass() -> bass.Bass:
    if "nc" not in _CACHE:
        _CACHE["nc"] = _build_bass()
    return _CACHE["nc"]


def _prep_core_inputs(C, Q, qmask, w, c):
    sl = slice(c * BL, (c + 1) * BL)
    Cc = C[sl]                                   # [BL, LC, D] f32
    # CTC[b, p, k, i] = C[b, i, 128k+p]
    CT = Cc.transpose(0, 2, 1).astype(BF)        # [BL, D, LC]
    CTC = np.ascontiguousarray(
        CT.reshape(BL, KD, 128, LC).transpose(0, 2, 1, 3)
    )                                            # [BL, 128, KD, LC]
    Qc = Q[sl].astype(BF)                        # [BL, LQ, D] bf16
    Qp = np.ascontiguousarray(Qc.transpose(1, 0, 2))   # [LQ, BL, D]
    QT = np.ascontiguousarray(
        Qc.transpose(2, 0, 1).reshape(KD, 128, BL, LQ).transpose(1, 0, 2, 3)
    )                                                  # [128, KD, BL, LQ]
    return {
        "CTC": CTC,
        "Qb": Qp,
        "QT": QT,
        "qmask": np.ascontiguousarray(qmask[sl], dtype=np.float32),
        "w": np.ascontiguousarray(w, dtype=np.float32),
    }


def _run(C, Q, qmask, w, trace=False, **spmd_kwargs):
    nc = _get_bass()
    C = np.ascontiguousarray(C, dtype=np.float32)
    Q = np.ascontiguousarray(Q, dtype=np.float32)
    qmask = np.ascontiguousarray(qmask, dtype=np.float32)
    w = np.ascontiguousarray(w, dtype=np.float32)
    in_maps = [_prep_core_inputs(C, Q, qmask, w, c) for c in range(N_CORES)]
    res = run_bass_kernel_spmd(
        nc, in_maps, list(range(N_CORES)), trace=trace, **spmd_kwargs
    )
    outT = np.concatenate(
        [np.asarray(res.results[c]["outT"]) for c in range(N_CORES)], axis=0
    )  # [B, 128, 4, LC] bf16: [b, p, (a k | ca k), i]
    # out[b, i, 128k+p (+256 for ca)] = outT[b, p, g, i]
    out = np.ascontiguousarray(
        outT.astype(np.float32).transpose(0, 3, 2, 1).reshape(B, LC, 2 * D)
    )
    return out, res


def kernel(C, Q, cmask, qmask, w):
    out, _ = _run(C, Q, qmask, w, trace=False)
    return out


# revision 10
# speedup vs baseline: 1.4158x; 1.0756x over previous
"""CQAttention (QANet context-query attention) Trainium2 kernel, v6.1.

Full-input contract: kernel(**inputs) takes the unsharded arrays
  C [64, 1024, 256] f32, Q [64, 128, 256] f32,
  cmask [64, 1024] f32 (unused by the reference), qmask [64, 128] f32,
  w [768] f32
and returns out [64, 1024, 512] f32.

Sharding: batch dim across 8 NeuronCores (8 batches/core), no
cross-core communication.

Design (history: v5 f32-I/O 91us, v6.0 73.6us):
  - All big DRAM I/O in bf16 (rel-err gate 2e-2; bf16 adds ~0.4%).
  - Everything computed transposed: host supplies C^T and Q^T packed so
    the kernel never transposes on-chip; output leaves as [2D, LC]^T and
    the host transposes back (layout only, no host FLOPs).
  - DMA queues are packet-rate-limited (~170ns/packet/DGE-engine
    measured), so host packs arrays to give 4KB (loads) / 8KB (store)
    contiguous runs per partition, and the whole batch output goes out
    in ONE dma_start.
  - Per batch: S^T = (w3*Q)^T @ C^T (4 mm) -> exp+bias (2 ACT) ->
    d = ones@E replicated across partitions (2 mm) ->
    r = reciprocal_approx_fast (1 DVE) -> En = E*r (DVE+POOL) ->
    A^T = Q @ En (4 mm, normalized) -> evac copies (4 ACT) +
    CA = u*C^T from PSUM (3 DVE) / from SBUF (1 POOL).
  - A-phase lags one iteration so the PE never waits on recip/Enorm.
    PE order per iter: d(b), S(b+1), A(b-1).
"""

from contextlib import ExitStack

import numpy as np
import ml_dtypes

import concourse.bacc as bacc
import concourse.bass as bass
import concourse.mybir as mybir
import concourse.tile as tile
from concourse.bass_utils import run_bass_kernel_spmd
from concourse.masks import make_identity

B, LC, LQ, D = 64, 1024, 128, 256
N_CORES = 8
BL = B // N_CORES  # batches per core
KD = D // 128      # d-chunks
F32 = mybir.dt.float32
BF16 = mybir.dt.bfloat16
MULT = mybir.AluOpType.mult
ADD = mybir.AluOpType.add
EXP = mybir.ActivationFunctionType.Exp
BF = ml_dtypes.bfloat16

_CACHE: dict = {}


def _build_bass() -> bass.Bass:
    nc = bacc.Bacc("TRN2")
    # CTC[b, p, k, i] = C[b, i, 128k+p] * nothing (raw C^T, packed so each
    # partition reads 4KB contiguous).
    CT_h = nc.dram_tensor("CTC", [BL, 128, KD, LC], BF16, kind="ExternalInput")
    Q_h = nc.dram_tensor("Qb", [LQ, BL, D], BF16, kind="ExternalInput")
    QT_h = nc.dram_tensor("QT", [128, KD, BL, LQ], BF16, kind="ExternalInput")
    qm_h = nc.dram_tensor("qmask", [BL, LQ], F32, kind="ExternalInput")
    w_h = nc.dram_tensor("w", [3 * D], F32, kind="ExternalInput")
    # OTP[b, p, k, c, i]: c in {A, CA} per d-chunk k; per-k stores are
    # 4KB/partition contiguous runs.
    out_h = nc.dram_tensor("outT", [BL, 128, KD, 2, LC], BF16, kind="ExternalOutput")

    with tile.TileContext(nc) as tc, ExitStack() as ctx:
        singles = ctx.enter_context(tc.tile_pool(name="singles", bufs=1))
        ct_pool = ctx.enter_context(tc.tile_pool(name="ct", bufs=5))
        e_pool = ctx.enter_context(tc.tile_pool(name="e", bufs=3))
        en_pool = ctx.enter_context(tc.tile_pool(name="en", bufs=2))
        r_pool = ctx.enter_context(tc.tile_pool(name="r", bufs=2))
        o_pool = ctx.enter_context(tc.tile_pool(name="o", bufs=3))
        small_pool = ctx.enter_context(tc.tile_pool(name="small", bufs=12))
        scratch_pool = ctx.enter_context(tc.tile_pool(name="scr", bufs=2))
        s_pool = ctx.enter_context(tc.tile_pool(name="s", bufs=2, space="PSUM"))
        u_pool = ctx.enter_context(tc.tile_pool(name="u", bufs=2, space="PSUM"))

        # ---------------- one-time setup ----------------
        ident32 = singles.tile([128, 128], F32)
        make_identity(nc, ident32)
        allones = singles.tile([128, 128], BF16)
        nc.vector.memset(allones, 1.0)
        one1 = singles.tile([1, 1], F32)
        nc.vector.memset(one1, 1.0)
        ones_row = singles.tile([1, 128], F32)
        nc.vector.memset(ones_row, 1.0)

        w_row = singles.tile([1, 3 * D], F32)
        nc.sync.dma_start(
            out=w_row, in_=bass.AP(tensor=w_h, offset=0, ap=[[1, 1], [1, 3 * D]])
        )
        qm8 = singles.tile([BL, LQ], F32)
        nc.sync.dma_start(
            out=qm8, in_=bass.AP(tensor=qm_h, offset=0, ap=[[LQ, BL], [1, LQ]])
        )
        qt_all = singles.tile([128, KD, BL, LQ], BF16)
        nc.sync.dma_start(
            out=qt_all,
            in_=bass.AP(
                tensor=QT_h,
                offset=0,
                ap=[[KD * BL * LQ, 128], [BL * LQ, KD], [1, BL * LQ]],
            ),
        )
        q_all = singles.tile([128, BL, D], BF16)
        nc.sync.dma_start(
            out=q_all,
            in_=bass.AP(
                tensor=Q_h, offset=0, ap=[[BL * D, 128], [D, BL], [1, D]]
            ),
        )

        c_tiles = [None] * BL

        def load_ct(b):
            ct_t = ct_pool.tile([128, KD, LC], BF16, name="ct")
            nc.sync.dma_start(out=ct_t, in_=CT_h[b])
            c_tiles[b] = ct_t

        load_ct(0)
        load_ct(1)
        load_ct(2)

        # w3T[p, k] = w[2D + 128k + p]; w2rep[p, :] = w2 broadcast.
        wps = s_pool.tile([128, KD + D], F32, name="wps", tag="s")
        for k in range(KD):
            nc.tensor.matmul(
                wps[:, k : k + 1],
                w_row[:, 2 * D + 128 * k : 2 * D + 128 * (k + 1)],
                one1,
                start=True,
                stop=True,
            )
        nc.tensor.matmul(
            wps[:, KD:], ones_row, w_row[:, D : 2 * D], start=True, stop=True
        )
        w3T = singles.tile([128, KD], F32)
        nc.vector.tensor_copy(out=w3T, in_=wps[:, :KD])
        w2rep = singles.tile([128, D], F32)
        nc.vector.tensor_copy(out=w2rep, in_=wps[:, KD:])

        qmT_ps = u_pool.tile([128, BL], F32, name="qmT_ps", tag="u")
        nc.tensor.matmul(qmT_ps, qm8, ident32[0:BL, 0:BL], start=True, stop=True)
        qmT = singles.tile([128, BL], F32)
        nc.vector.tensor_copy(out=qmT, in_=qmT_ps)

        # qtw3[p, k, b, j] = w3[128k+p] * Q^T[128k+p, b, j].
        # Batch 0 first (tiny, unblocks S(0)); batches 1.. in one bulk op
        # per k while the head of the pipeline is otherwise idle on DVE.
        qtw3 = singles.tile([128, KD, BL, LQ], BF16)

        def qtw3_prep(b_slice, eng):
            for k in range(KD):
                eng.tensor_scalar_mul(
                    out=qtw3[:, k, b_slice],
                    in0=qt_all[:, k, b_slice],
                    scalar1=w3T[:, k : k + 1],
                )

        bias_all = singles.tile([128, BL], F32)

        def qprep(b):
            q2sb = small_pool.tile([128, 1], F32, name="q2sb")
            scr = scratch_pool.tile([128, D], F32, name="scr")
            nc.vector.scalar_tensor_tensor(
                out=scr,
                in0=q_all[:, b],
                scalar=1.0,
                in1=w2rep,
                op0=MULT,
                op1=MULT,
                accum_out=q2sb,
            )
            nc.vector.scalar_tensor_tensor(
                out=bias_all[:, b : b + 1],
                in0=qmT[:, b : b + 1],
                scalar=-10000.0,
                in1=q2sb,
                op0=MULT,
                op1=ADD,
            )

        # ---------------- per-batch pipeline stages ----------------
        def stage_s(b):
            ct_t = c_tiles[b]
            s_t = s_pool.tile([128, 2, 512], F32, name="s_t", tag="s")
            for h in range(2):
                for k in range(KD):
                    nc.tensor.matmul(
                        s_t[:, h],
                        qtw3[:, k, b],
                        ct_t[:, k, 512 * h : 512 * (h + 1)],
                        start=(k == 0),
                        stop=(k == KD - 1),
                    )
            e_t = e_pool.tile([128, 2, 512], BF16)
            nc.scalar.activation(
                out=e_t, in_=s_t, func=EXP,
                bias=bias_all[:, b : b + 1], scale=1.0,
            )
            return e_t

        def stage_d_mm(b, e_t):
            d_ps = s_pool.tile([128, 2, 512], F32, name="d_ps", tag="s")
            for h in range(2):
                nc.tensor.matmul(
                    d_ps[:, h], allones, e_t[:, h], start=True, stop=True
                )
            return d_ps

        def stage_d_vec(b, e_t, d_ps):
            """Per-half reciprocal; half 1 of E is pre-normalized on POOL
            (half 0 gets its 1/d folded into the A-evacuation on DVE)."""
            r_t = r_pool.tile([128, 2, 512], F32)
            for h in range(2):
                nc.vector.reciprocal_approx_fast(out=r_t[:, h], in_=d_ps[:, h])
            en1_t = en_pool.tile([128, 512], BF16)
            nc.gpsimd.tensor_tensor(
                out=en1_t, in0=e_t[:, 1], in1=r_t[:, 1], op=MULT
            )
            return (e_t, en1_t, r_t)

        def stage_a(b, en):
            """A^T matmuls. Half 0: rhs is raw E, 1/d folded into the DVE
            evacuation (tt from PSUM). Half 1: rhs pre-normalized, plain ACT
            copy evac. CA: half 0 on POOL, half 1 on DVE (both SBUF bf16)."""
            e_t, en1_t, r_t = en
            ct_t = c_tiles[b]
            o_t = o_pool.tile([128, KD, 2, LC], BF16)
            for k in range(KD):
                u_t = u_pool.tile([128, 2, 512], F32, name="u_t", tag="u")
                lhs = q_all[:, b, 128 * k : 128 * (k + 1)]
                nc.tensor.matmul(u_t[:, 0], lhs, e_t[:, 0], start=True, stop=True)
                nc.tensor.matmul(u_t[:, 1], lhs, en1_t, start=True, stop=True)
                sl0 = slice(0, 512)
                sl1 = slice(512, 1024)
                # evac h0: fold 1/d (DVE tt, PSUM x SBUF f32)
                nc.vector.tensor_tensor(
                    out=o_t[:, k, 0, sl0], in0=u_t[:, 0], in1=r_t[:, 0], op=MULT
                )
                # evac h1: plain copy (ACT)
                nc.scalar.copy(out=o_t[:, k, 0, sl1], in_=u_t[:, 1])
                # CA h0 on POOL (SBUF bf16), CA h1 on DVE (SBUF bf16)
                nc.gpsimd.tensor_tensor(
                    out=o_t[:, k, 1, sl0], in0=o_t[:, k, 0, sl0],
                    in1=ct_t[:, k, sl0], op=MULT,
                )
                nc.vector.tensor_tensor(
                    out=o_t[:, k, 1, sl1], in0=o_t[:, k, 0, sl1],
                    in1=ct_t[:, k, sl1], op=MULT,
                )
            return o_t

        def store_o(b, o_t):
            for k in range(KD):
                nc.sync.dma_start(out=out_h[b, :, k], in_=o_t[:, k])

        # ---------------- software-pipelined emission ----------------
        qprep(0)
        qtw3_prep(slice(0, 1), nc.vector)
        qprep(1)
        e_tiles = {0: stage_s(0)}
        qtw3_prep(slice(1, BL), nc.vector)  # bulk, off the critical path
        d_cur = stage_d_mm(0, e_tiles[0])
        en_tiles = {0: stage_d_vec(0, e_tiles[0], d_cur)}
        pending_store = None
        for b in range(BL + 1):
            if b + 3 < BL:
                load_ct(b + 3)
            if pending_store is not None:
                store_o(*pending_store)
                pending_store = None
            # PE order: d(b) first (inputs long ready), then S(b+1), then
            # A(b-1); recip/Enorm(b) start right after d(b).
            if 1 <= b < BL:
                d_ps = stage_d_mm(b, e_tiles[b])
                r_en = (b, e_tiles.pop(b), d_ps)
            else:
                r_en = None
            if b + 1 < BL:
                e_tiles[b + 1] = stage_s(b + 1)
            if r_en is not None:
                en_tiles[b] = stage_d_vec(*r_en)
            if b >= 1:
                pending_store = (b - 1, stage_a(b - 1, en_tiles.pop(b - 1)))
            if b + 2 < BL:
                qprep(b + 2)
        store_o(*pending_store)
    nc.compile()
    return nc


def _get_bass() -> bass.Bass:
    if "nc" not in _CACHE:
        _CACHE["nc"] = _build_bass()
    return _CACHE["nc"]


def _prep_core_inputs(C, Q, qmask, w, c):
    sl = slice(c * BL, (c + 1) * BL)
    Cc = C[sl]                                   # [BL, LC, D] f32
    # CTC[b, p, k, i] = C[b, i, 128k+p]
    CT = Cc.transpose(0, 2, 1).astype(BF)        # [BL, D, LC]
    CTC = np.ascontiguousarray(
        CT.reshape(BL, KD, 128, LC).transpose(0, 2, 1, 3)
    )                                            # [BL, 128, KD, LC]
    Qc = Q[sl].astype(BF)                        # [BL, LQ, D] bf16
    Qp = np.ascontiguousarray(Qc.transpose(1, 0, 2))   # [LQ, BL, D]
    QT = np.ascontiguousarray(
        Qc.transpose(2, 0, 1).reshape(KD, 128, BL, LQ).transpose(1, 0, 2, 3)
    )                                                  # [128, KD, BL, LQ]
    return {
        "CTC": CTC,
        "Qb": Qp,
        "QT": QT,
        "qmask": np.ascontiguousarray(qmask[sl], dtype=np.float32),
        "w": np.ascontiguousarray(w, dtype=np.float32),
    }


def _run(C, Q, qmask, w, trace=False, **spmd_kwargs):
    nc = _get_bass()
    C = np.ascontiguousarray(C, dtype=np.float32)
    Q = np.ascontiguousarray(Q, dtype=np.float32)
    qmask = np.ascontiguousarray(qmask, dtype=np.float32)
    w = np.ascontiguousarray(w, dtype=np.float32)
    in_maps = [_prep_core_inputs(C, Q, qmask, w, c) for c in range(N_CORES)]
    res = run_bass_kernel_spmd(
        nc, in_maps, list(range(N_CORES)), trace=trace, **spmd_kwargs
    )
    outT = np.concatenate(
        [np.asarray(res.results[c]["outT"]) for c in range(N_CORES)], axis=0
    )  # [B, 128, KD, 2, LC] bf16: [b, p, k, (a|ca), i]
    # out[b, i, 256*c + 128*k + p] = outT[b, p, k, c, i]
    out = np.ascontiguousarray(
        outT.astype(np.float32).transpose(0, 4, 3, 2, 1).reshape(B, LC, 2 * D)
    )
    return out, res


def kernel(C, Q, cmask, qmask, w):
    out, _ = _run(C, Q, qmask, w, trace=False)
    return out
